# revision 22
# baseline (speedup 1.0000x reference)
"""CrossAttention1D Trainium2 kernel (fp8 DoubleRow edition).

Problem: B=4, C=1024, L=2048, H=16 heads (D=64). LKV == LQ so the
reference's linear interpolation is the identity and is skipped.

Sharding (8 cores): data-parallel over batch (4) x tensor-parallel over
heads (2 halves of 8 heads). Core c handles batch c//2, heads
(c%2)*8 .. (c%2)*8+8. Each core computes its half of Q/K/V projections
(512 of 1024 channels), attention for its 8 heads, and a partial output
projection Wo[:, shard] @ O (+ residual/bias on even cores). The host
sums the two partials per batch.

Device dataflow per core — everything runs as fp8e4m3 DoubleRow matmuls
(2 k-subtiles packed per instruction, 2x PE throughput):
  * Projections contract C=1024 as 4 DoubleRow matmuls.
  * QK^T splits D=64 into 2x32: Wq/Wk columns are host-permuted so the
    projection output lands as [d0:32 | d32:64] k-subtiles on 32
    partitions per head; the two heads of a pair run concurrently in
    32-row PE tile groups.
  * Softmax is exp(S/8 - 3) (shift-invariant, keeps P in fp8 range)
    written by ACT directly to fp8; a ones-column in the V stationary
    accumulates the denominator row during the AV DoubleRow.
  * Per head pair, raw O is staged to SBUF (releasing PSUM
    accumulators), denominators inverted on DVE, and a selector-matmul
    broadcast + fused multiply produce fp8 O for the Wo DoubleRow.
All non-attention work (remaining projections, normalization, Wo with
prefetched residuals) is interleaved into the attention jp loop as
slot-gated PE filler so ACT (the roofline engine at ~1us per 128x1024
exp tile) never waits.
"""

import json

import numpy as np
import ml_dtypes

import concourse.bass as bass
import concourse.mybir as mybir
import concourse.tile as tile
from concourse.bass_utils import run_bass_kernel_spmd

BF16 = ml_dtypes.bfloat16
F8 = ml_dtypes.float8_e4m3

B, C, L, H, D = 4, 1024, 2048, 16, 64
CS = C // 2          # channel shard per core (512)
HPC = H // 2         # heads per core (8)
NCORES = 8
SCALE = 1.0 / np.sqrt(D)  # 0.125
EXP_BIAS = -3.0      # exp(S*SCALE + EXP_BIAS): softmax-invariant shift
VP = 80              # per-head stride in V_sb (65 used, 16B-aligned)

_DT = mybir.dt

_MAX_WAITS = 1


def _split_drain_waits(nc):
    """Hoist excess per-instruction sync-waits onto preceding NoOps.

    This toolchain's walrus codegen rejects instructions carrying more
    than one sync wait ("Too many sync wait commands"). Hoisting a wait
    onto a NoOp immediately before the instruction on the same engine is
    semantics-preserving (engines execute their stream in order).
    """
    j = json.loads(nc.to_json_bytes())
    n_hoisted = 0
    for fn in j["functions"]:
        for bb in fn["blocks"]:
            out = []
            for inst in bb["instructions"]:
                si = inst.get("sync_info")
                ow = (si or {}).get("on_wait") or []
                if len(ow) > _MAX_WAITS:
                    n_hoisted += 1
                    for i, w in enumerate(ow[: -_MAX_WAITS]):
                        out.append(
                            {
                                "engine": inst["engine"],
                                "ins": [],
                                "outs": [],
                                "name": f"{inst['name']}_hw{i}",
                                "opcode": "NoOp",
                                "debug": inst.get("debug"),
                                "sync_info": {"on_update": [], "on_wait": [w]},
                            }
                        )
                    si["on_wait"] = ow[-_MAX_WAITS:]
                out.append(inst)
            bb["instructions"] = out
    patched = json.dumps(j).encode()
    nc.to_json_bytes = lambda: patched
    return nc


def _build_nc():
    nc = bass.Bass()
    dt = _DT
    bf = dt.bfloat16
    f8 = dt.float8e4
    f32 = dt.float32
    DR = mybir.MatmulPerfMode.DoubleRow

    q_d = nc.declare_dram_parameter("q8", [C, L], f8, isOutput=False)
    ctx_d = nc.declare_dram_parameter("ctx8", [C, L], f8, isOutput=False)
    wq_d = nc.declare_dram_parameter("wqT", [C, CS], f8, isOutput=False)
    wk_d = nc.declare_dram_parameter("wkT", [C, CS], f8, isOutput=False)
    wv_d = nc.declare_dram_parameter("wvT", [C, CS], f8, isOutput=False)
    wo_d = nc.declare_dram_parameter("woT", [CS, C], f8, isOutput=False)
    res_d = nc.declare_dram_parameter("resid", [C, L], f32, isOutput=False)
    selm_d = nc.declare_dram_parameter(
        "selm", [HPC, HPC * 64], bf, isOutput=False
    )
    out_d = nc.declare_dram_parameter("out", [C, L], f32, isOutput=True)

    KT = C // 128        # 8 contraction tiles for projections
    KP = KT // 2         # 4 DoubleRow k-pairs
    CT = CS // 128       # 4 channel tiles of the shard
    CP = CT // 2         # 2 DoubleRow k-pairs for Wo
    LT = L // 512        # 4 L-tiles of 512
    JT = L // 128        # 16 j-tiles of 128
    JP = JT // 2         # 8 j-pairs (DoubleRow AV)

    with tile.TileContext(nc) as tc:
        with (
            tc.tile_pool(name="const", bufs=1) as cp,
            tc.tile_pool(name="pwork", bufs=3) as pwork,
            tc.tile_pool(name="norm", bufs=2) as normp,
            tc.tile_pool(name="io", bufs=3) as iop,
            tc.tile_pool(name="psum", bufs=3, space="PSUM") as psp,
        ):
            # ---- resident SBUF slabs
            q_sb = cp.tile([128, KT, L], f8)       # query, c_in on partitions
            c_sb = cp.tile([128, KT, L], f8)       # context
            wq_sb = cp.tile([128, KT, CS], f8)
            wk_sb = cp.tile([128, KT, CS], f8)
            wv_sb = cp.tile([128, KT, CS], f8)
            wo_sb = cp.tile([128, CT, C], f8)
            # Q8/K8: [p, pair-group, k-subtile, L]; pair tp sits at
            # partitions (tp%2)*64..+64 of group tp//2, head a (d 0:32 /
            # 32:64 as the 2 k-subtiles) at +0:32, head b at +32:64.
            Q8_sb = cp.tile([128, 2, 2, L], f8)
            K8_sb = cp.tile([128, 2, 2, L], f8)
            V_sb = cp.tile([128, JT, HPC * VP], f8)  # V^T + ones cols, padded
            O_sb = cp.tile([128, CT, L], f8)       # normalized attn output

            # single multi-dim DMA per input tensor: the Sync engine takes
            # ~0.65us to *issue* each DMA, so per-k-tile loads serialize.
            nc.sync.dma_start(c_sb, ctx_d.rearrange("(k p) m -> p k m", p=128))
            nc.sync.dma_start(wk_sb, wk_d.rearrange("(k p) m -> p k m", p=128))
            nc.sync.dma_start(wv_sb, wv_d.rearrange("(k p) m -> p k m", p=128))
            nc.sync.dma_start(q_sb, q_d.rearrange("(k p) m -> p k m", p=128))
            nc.sync.dma_start(wq_sb, wq_d.rearrange("(k p) m -> p k m", p=128))
            nc.sync.dma_start(wo_sb, wo_d.rearrange("(k p) m -> p k m", p=128))

            # ones columns for the AV denominator rows
            v_view = V_sb.rearrange("p j (h e) -> p j h e", e=VP)
            for jt in range(JT):
                nc.vector.memset(v_view[:, jt, :, D : D + 1], 1.0)
            # selm[p, h*64:(h+1)*64] = 1 iff p == h: lhsT selector that
            # broadcasts row h of an [HPC, N] rhs onto 64 output partitions.
            selm = cp.tile([HPC, HPC * 64], bf)
            nc.sync.dma_start(selm, selm_d[:, :])
            bias_sb = cp.tile([128, 1], f32)
            nc.vector.memset(bias_sb, EXP_BIAS)

            # ---- emission helpers
            def emit_proj_qk8(dst8, w_sb, src, ct, lt):
                # fp8 projection with host-permuted weight columns; output
                # partitions are [a.d0:32 | b.d0:32 | a.d32:64 | b.d32:64]
                # so two plain copies land the d-split k-subtile layout.
                ls = slice(lt * 512, (lt + 1) * 512)
                p = psp.tile([128, 512], f32, tag="s")
                for kp in range(KP):
                    nc.tensor.matmul(
                        p,
                        lhsT=w_sb[:, 2 * kp : 2 * kp + 2, ct * 128 : (ct + 1) * 128],
                        rhs=src[:, 2 * kp : 2 * kp + 2, ls],
                        start=(kp == 0),
                        stop=(kp == KP - 1),
                        perf_mode=DR,
                    )
                pb = (ct % 2) * 64
                nc.vector.tensor_copy(
                    dst8[pb : pb + 64, ct // 2, 0, ls], p[0:64, :]
                )
                nc.vector.tensor_copy(
                    dst8[pb : pb + 64, ct // 2, 1, ls], p[64:128, :]
                )

            def emit_proj_v(jt):
                pv = psp.tile([128, 512], f32, tag="s")
                for kp in range(KP):
                    nc.tensor.matmul(
                        pv,
                        lhsT=c_sb[:, 2 * kp : 2 * kp + 2, jt * 128 : (jt + 1) * 128],
                        rhs=wv_sb[:, 2 * kp : 2 * kp + 2, :],
                        start=(kp == 0),
                        stop=(kp == KP - 1),
                        perf_mode=DR,
                    )
                nc.vector.tensor_copy(
                    v_view[:, jt, :, 0:D],
                    pv.rearrange("p (h d) -> p h d", d=D),
                )

            def emit_wo(state, mt):
                it_prev = state["it"]
                psl = slice(it_prev * 512, (it_prev + 1) * 512)
                po = psp.tile([128, 512], f32, tag="s")
                for kp in range(CP):
                    nc.tensor.matmul(
                        po,
                        lhsT=wo_sb[:, 2 * kp : 2 * kp + 2, mt * 128 : (mt + 1) * 128],
                        rhs=O_sb[:, 2 * kp : 2 * kp + 2, psl],
                        start=(kp == 0),
                        stop=(kp == CP - 1),
                        perf_mode=DR,
                    )
                ot = iop.tile([128, 512], f32, tag="out")
                nc.vector.tensor_add(ot, po, state["rt"][:, mt, :])
                nc.sync.dma_start(out_d[mt * 128 : (mt + 1) * 128, psl], ot)

            def emit_norm(state, hh):
                # PE broadcast of 1/denom row + fused multiply to fp8 O_sb
                it_prev, row = state["it"], hh % 2
                rb_ps = psp.tile([64, 512], f32, tag="s")
                nc.tensor.matmul(
                    rb_ps,
                    lhsT=selm[0:2, row * 64 : (row + 1) * 64],
                    rhs=state["recb"],
                    start=True, stop=True,
                )
                otmp = normp.tile([64, 512], f8, tag="otmp")
                nc.vector.tensor_mul(otmp, state["oraw"][hh][0:D, :], rb_ps)
                poff = (hh % 2) * 64
                psl = slice(it_prev * 512, (it_prev + 1) * 512)
                nc.sync.dma_start(O_sb[poff : poff + 64, hh // 2, psl], otmp)

            # ---- upfront projections: K ct0, V j-tiles 0..3, Q i-tile 0.
            # The rest streams in as slot-gated PE filler.
            for lt in range(LT):
                emit_proj_qk8(K8_sb, wk_sb, c_sb, 0, lt)
            for jt in range(4):
                emit_proj_v(jt)
            for ct in range(CT):
                emit_proj_qk8(Q8_sb, wq_sb, q_sb, ct, 0)

            # ---- attention epochs with slot-gated filler
            Exp = mybir.ActivationFunctionType.Exp
            DEPTH = 2
            SLOTS = JP + DEPTH  # per head pair

            pending = []  # filler carried into the next epoch

            for it in range(LT):
                isl = slice(it * 512, (it + 1) * 512)
                oraw = [
                    normp.tile([D + 1, 512], bf, tag=f"oraw{h}",
                               name=f"oraw{h}", bufs=2)
                    for h in range(HPC)
                ]
                filler = pending
                pending = []
                if it == 0:
                    filler += [(0, "v", None, jt) for jt in range(4, JT)]
                    filler += [
                        (0, "k", None, (ct, lt))
                        for ct in range(1, CT)
                        for lt in range(LT)
                    ]
                if it < LT - 1:
                    filler += [(0, "q", None, ct) for ct in range(CT)]

                def emit_item(kind, state, arg):
                    if kind == "norm":
                        emit_norm(state, arg)
                    elif kind == "wo":
                        emit_wo(state, arg)
                    elif kind == "v":
                        emit_proj_v(arg)
                    elif kind == "k":
                        emit_proj_qk8(K8_sb, wk_sb, c_sb, arg[0], arg[1])
                    else:
                        emit_proj_qk8(Q8_sb, wq_sb, q_sb, arg, it + 1)

                def do_filler(slot, budget=2):
                    while budget > 0 and filler and filler[0][0] <= slot:
                        _, kind, state, arg = filler.pop(0)
                        emit_item(kind, state, arg)
                        budget -= 1 if kind in ("v", "k") else 2

                def force_items(match):
                    # correctness net: anything the upcoming instructions
                    # depend on must already be in the PE stream.
                    rest = []
                    for f in filler:
                        if match(f):
                            emit_item(f[1], f[2], f[3])
                        else:
                            rest.append(f)
                    filler[:] = rest

                for tp in range(CT):  # head pair (2*tp, 2*tp+1)
                    if it == 0 and tp > 0:
                        force_items(
                            lambda f: f[1] == "k" and f[3][0] <= tp
                        )
                    pb = (tp % 2) * 64
                    pg = tp // 2
                    pOa = psp.tile([D + 1, 512], f32, tag="acc", bufs=2)
                    pOb = psp.tile([D + 1, 512], f32, tag="acc", bufs=2)
                    pend = []
                    for jp in range(SLOTS):
                        if jp < JP:
                            Pab = pwork.tile([128, 2, 1024], f8, tag="p")
                            for t in range(2):
                                jt = 2 * jp + t
                                js = slice(jt * 128, (jt + 1) * 128)
                                pS = psp.tile([128, 1024], f32, tag="s")
                                nc.tensor.matmul(
                                    pS[:, 0:512],
                                    lhsT=K8_sb[pb : pb + 32, pg, 0:2, js],
                                    rhs=Q8_sb[pb : pb + 32, pg, 0:2, isl],
                                    start=True,
                                    stop=True,
                                    perf_mode=DR,
                                    tile_position=(pb, 0),
                                )
                                nc.tensor.matmul(
                                    pS[:, 512:1024],
                                    lhsT=K8_sb[pb + 32 : pb + 64, pg, 0:2, js],
                                    rhs=Q8_sb[pb + 32 : pb + 64, pg, 0:2, isl],
                                    start=True,
                                    stop=True,
                                    perf_mode=DR,
                                    tile_position=(pb + 32, 0),
                                )
                                nc.scalar.activation(
                                    Pab[:, t, :], pS, Exp,
                                    bias=bias_sb[:, :], scale=SCALE,
                                )
                            pend.append((Pab, jp))
                        if len(pend) > (DEPTH if jp < JP else 0):
                            if it == 0 and tp == 0:
                                force_items(
                                    lambda f: f[1] == "v"
                                    and f[3] <= 2 * pend[0][1] + 3
                                )
                            Pab_r, qjp = pend.pop(0)
                            ha, hb = 2 * tp, 2 * tp + 1
                            nc.tensor.matmul(
                                pOa,
                                lhsT=V_sb[:, 2 * qjp : 2 * qjp + 2,
                                          ha * VP : ha * VP + D + 1],
                                rhs=Pab_r[:, 0:2, 0:512],
                                start=(qjp == 0),
                                stop=(qjp == JP - 1),
                                perf_mode=DR,
                            )
                            nc.tensor.matmul(
                                pOb,
                                lhsT=V_sb[:, 2 * qjp : 2 * qjp + 2,
                                          hb * VP : hb * VP + D + 1],
                                rhs=Pab_r[:, 0:2, 512:1024],
                                start=(qjp == 0),
                                stop=(qjp == JP - 1),
                                perf_mode=DR,
                            )
                        do_filler(tp * SLOTS + jp)

                    # stage both heads' raw O (+denominator row) to SBUF,
                    # releasing the PSUM accumulators; invert the pair's
                    # denominators on DVE; normalization multiplies join the
                    # filler stream ~1.5 pairs later (recb latency ~5us).
                    dn2 = normp.tile([2, 512], bf, tag="dn", bufs=2)
                    for i2, pO in ((0, pOa), (1, pOb)):
                        nc.vector.tensor_copy(oraw[2 * tp + i2], pO)
                        nc.sync.dma_start(
                            dn2[i2 : i2 + 1, :],
                            oraw[2 * tp + i2][D : D + 1, :],
                        )
                    recf = normp.tile([2, 512], f32, tag="recf", bufs=2)
                    nc.vector.reciprocal(recf, dn2)
                    recb = normp.tile([2, 512], bf, tag="recb", bufs=4)
                    nc.vector.tensor_copy(recb, recf)
                    state = {"recb": recb, "oraw": oraw, "it": it}
                    filler += [
                        ((tp + 1) * SLOTS + 5, "norm", state, 2 * tp + i2)
                        for i2 in range(2)
                    ]

                # leftovers (pair-3 norms and any stragglers) carry over,
                # keeping their slot phase relative to the new epoch
                for f in filler:
                    pending.append((max(0, f[0] - CT * SLOTS), f[1], f[2], f[3]))
                # Wo for this epoch runs in the next one; prefetch residuals
                # now (one slab DMA) so the adds never wait.
                rt = iop.tile([128, C // 128, 512], f32, tag="rt", bufs=2)
                nc.sync.dma_start(
                    rt, res_d[:, isl].rearrange("(m p) l -> p m l", p=128)
                )
                wstate = {"rt": rt, "it": it}
                pending += [
                    (8 + 2 * mt, "wo", wstate, mt) for mt in range(C // 128)
                ]

            # final flush: pair-3 norms of the last epoch + its Wo batch
            filler = pending
            while filler:
                _, kind, state, arg = filler.pop(0)
                if kind == "norm":
                    emit_norm(state, arg)
                else:
                    emit_wo(state, arg)
    return nc


_NC = None


def _get_nc():
    global _NC
    if _NC is None:
        _NC = _split_drain_waits(_build_nc())
    return _NC


def _qk_col_perm():
    # within each 128-col tile: [a.d0:32 | a.d32:64 | b.d0:32 | b.d32:64]
    # -> [a.d0:32 | b.d0:32 | a.d32:64 | b.d32:64]
    perm = []
    for ct in range(CT := CS // 128):
        base = ct * 128
        perm += list(range(base, base + 32))
        perm += list(range(base + 64, base + 96))
        perm += list(range(base + 32, base + 64))
        perm += list(range(base + 96, base + 128))
    return np.array(perm)


def _make_in_maps(query, context, Wq, Wk, Wv, Wo, bo):
    zeros_res = np.zeros((C, L), np.float32)
    selm = np.zeros((HPC, HPC * 64), dtype=BF16)
    for h in range(HPC):
        selm[h, h * 64 : (h + 1) * 64] = 1.0
    perm = _qk_col_perm()
    in_maps = []
    for c in range(NCORES):
        b, hf = c // 2, c % 2
        rows = slice(hf * CS, (hf + 1) * CS)
        in_maps.append(
            {
                "q8": query[b].astype(F8),
                "ctx8": context[b].astype(F8),
                "wqT": np.ascontiguousarray(Wq[rows].T[:, perm]).astype(F8),
                "wkT": np.ascontiguousarray(Wk[rows].T[:, perm]).astype(F8),
                "wvT": np.ascontiguousarray(Wv[rows].T).astype(F8),
                "woT": np.ascontiguousarray(Wo[:, rows].T).astype(F8),
                "resid": (query[b] + bo[:, None]).astype(np.float32)
                if hf == 0
                else zeros_res,
                "selm": selm,
            }
        )
    return in_maps


def _gather(results):
    out = np.empty((B, C, L), np.float32)
    for b in range(B):
        out[b] = results[2 * b]["out"] + results[2 * b + 1]["out"]
    return out


def kernel(query, context, Wq, Wk, Wv, Wo, bo, heads):
    query = np.asarray(query, dtype=np.float32)
    context = np.asarray(context, dtype=np.float32)
    Wq = np.asarray(Wq, dtype=np.float32)
    Wk = np.asarray(Wk, dtype=np.float32)
    Wv = np.asarray(Wv, dtype=np.float32)
    Wo = np.asarray(Wo, dtype=np.float32)
    bo = np.asarray(bo, dtype=np.float32)
    assert int(heads) == H
    assert query.shape == (B, C, L) and context.shape == (B, C, L)

    nc = _get_nc()
    in_maps = _make_in_maps(query, context, Wq, Wk, Wv, Wo, bo)
    res = run_bass_kernel_spmd(nc, in_maps, list(range(NCORES))).results
    return _gather(res)
